# revision 34
# baseline (speedup 1.0000x reference)
"""Trainium2 Bass kernel for nn_BiaffineNER (BiDAF attention + FFW + biaffine scorer).

Contract: kernel(**inputs) takes the FULL unsharded inputs (numpy) and returns
the FULL [16, 512, 512, 3] float32 output. Internally shards data-parallel over
the batch axis across 8 NeuronCores (2 batch elements per core), runs one SPMD
Bass/Tile program on all cores, and concatenates the per-core outputs.

Math per batch element b (LC=512, LQ=64, H=256, D=4H=1024, DFF=512, C=3):
  sim  = (ctx@w1)[:,None] + (ques@w2)[None,:] + (ctx*w3)@ques.T      [LC,LQ]
  a    = softmax_j(sim); c2q = a @ ques                              [LC,H]
  bwt  = softmax_i(max_j sim); q2c = bwt @ ctx                       [H]
  x    = [ctx, c2q, ctx*c2q, ctx*q2c]                                [LC,D]
  start= relu(x@W1s+b1s)@W2s+b2s ; end likewise                      [LC,D]
  out[x,y,c] = [start,1][x] . Wb[:,c,:] . [end,1][y]                 [LC,LC,C]

Design notes:
- Activations kept transposed on-chip ([feature-part, token-free]) so the
  contraction dim always sits on SBUF partitions; ctx^T/ques^T come in
  host-pretransposed.
- All matmul operands are bfloat16 (PE full rate; fp32 PSUM accumulate); rel
  err ~6e-3 vs 2e-2 tolerance.  fp8 DoubleRow was analyzed and rejected: e4m3
  mantissa noise is ~2.5-3.7%/stage on this data, over the error budget.
- All weights (FFW + the 12.6MB Wb) are loaded once in bf16 and stay resident
  in SBUF for both batch elements.
- DMA packets round-robin across ACTIVE queues, so tiny-row const DMAs get
  starved while bulk flows; the bf16 const pack is therefore EMBEDDED in the
  batch-0 ctxT|quesT pack (one coarse-packet DMA on the sync hw queue).  The
  ques|ctx packs ride the scalar queue, W1s/W2e the gpsimd software DGE
  (measured ~265GB/s), W1e/W2s/Wb the sync queue in first-use order.  The
  former 131KB all-ones pack block is an on-chip memset tile that doubles as
  the PE warm-up operand.
- Schedule: attention(b0) runs solo as soon as its pack lands, with a short
  warm-up burst + keepalive matmuls bridging the DMA wait; all four h1-s
  accumulation groups of FFW(b0) are PRE-STARTED with their ctxT and
  (ctx*q2c)^T terms (real dense work filling batch 0's serial attention
  tail), so the PE HAM clock gate (3413ns evaluation quanta, ~14.5us reopen
  hysteresis once dropped) opens early and stays open into the dense phase
  (measured: no mid-kernel K=4/8 re-throttle).  attention(b1)'s four stages
  are emitted inside hooks between FFW(b0)'s stage blocks so its vector/
  scalar work hides under FFW0's dense PE stream; FFW psum groups alternate
  between two pools (6 bufs of WAR slack) and evacuations alternate between
  the vector and scalar engines so the hooked work cannot stall the PE.
  Then FFW1, biaffine0, biaffine1 keep the PE saturated to the end.
- FFW h1 accumulation orders put the late-produced xT chunks (c2q, ctx*c2q)
  last, so each group can start before the attention tail finishes.
- Softmaxes skip max-subtraction (|sim| < ~8 for this data distribution).
- The output leaves as bf16 [C, LC, LC] planes (host transposes/upcasts),
  one DMA per (batch, label) plane group on the sync queue.
"""

import sys

if "/opt/trn_rl_repo" not in sys.path and "/root/.axon_site/_ro/trn_rl_repo" not in sys.path:
    sys.path.insert(0, "/opt/trn_rl_repo")

import numpy as np

import concourse.bass as bass
import concourse.tile as tile
from concourse import bacc, mybir

F32 = mybir.dt.float32
F16 = mybir.dt.bfloat16
AF = mybir.ActivationFunctionType
ALU = mybir.AluOpType
AX = mybir.AxisListType

N_CORES = 8
B, LC, LQ, H = 16, 512, 64, 256
NB = B // N_CORES          # batch elements per core
D, DFF, C = 4 * H, 512, 3
NIC = LC // 128            # 4  i/x chunks
NHC = H // 128             # 2  h chunks
NDC = D // 128             # 8  d chunks
NFC = DFF // 128           # 4  f chunks
NJC = 8                    # j chunks (first 1024 of 1025)

# per-batch input packs.  p1a (attention-critical, sync queue): ctxT | quesT
# | bf16 const pack (embedded so the whole critical set rides ONE coarse-
# packet DMA -- separate small-row const DMAs get starved by the per-packet
# round-robin across active queues).  p1b (scalar queue): ques | ctx.
OFF_CT = 0                 # 1024 cols: ctxT hc-major
OFF_QT = 1024              # 128 cols: quesT hc-major
OFF_BF = 1152              # 175 cols: bf16 const pack (batch 0 copy is used)
P1AW = 1327
OFF_Q = 0                  # 256 cols x rows 0:64: ques
OFF_CX = 256               # 1024 cols: ctx ic-major
P1BW = 1280

# f32 constant pack column layout [128, 50]
COL_W3 = 0                 # 2 cols (w3 chunks, per-partition scalars)
COL_B1S, COL_B2S = 2, 6    # 4 + 8
COL_B1E, COL_B2E = 14, 18  # 4 + 8
COL_VC = 26                # 24 cols: vcols[c*NJC + jc]
NF32 = 50

# bf16 constant pack column layout [128, 175]
OFF_COLW = 0               # 8 cols: [w1_0, 0, w1_1, 0, 0, w2_0, 0, w2_1]
OFF_IDENT = 8              # 128 cols identity
OFF_UPACK = 136            # 32 cols: upack[dc*4 + c] = Wb[dc*128+p, c, D]
OFF_WROW4 = 168            # 4 cols: Wb[D, c, D] broadcast down partitions
OFF_Z = 172                # 3 cols [1, 0, 1]: [z:z+2]=[1,0], [z+1:z+3]=[0,1]
NBF16 = 175


def _build_program():
    nc = bacc.Bacc("TRN2", target_bir_lowering=False, debug=False,
                   num_devices=N_CORES)

    p1a_d = nc.dram_tensor("p1a", [NB, 128, P1AW], F16, kind="ExternalInput").ap()
    p1b_d = nc.dram_tensor("p1b", [NB, 128, P1BW], F16, kind="ExternalInput").ap()
    f32p_d = nc.dram_tensor("f32pack", [128, NF32], F32, kind="ExternalInput").ap()
    wb_d = nc.dram_tensor("wb", [128, NDC, C, D + 1], F16, kind="ExternalInput").ap()
    w1s_d = nc.dram_tensor("W1s", [128, NDC, DFF], F16, kind="ExternalInput").ap()
    w2s_d = nc.dram_tensor("W2s", [128, NFC, D], F16, kind="ExternalInput").ap()
    w1e_d = nc.dram_tensor("W1e", [128, NDC, DFF], F16, kind="ExternalInput").ap()
    w2e_d = nc.dram_tensor("W2e", [128, NFC, D], F16, kind="ExternalInput").ap()
    out_d = nc.dram_tensor("out", [NB, C, 128, NIC, LC], F16, kind="ExternalOutput").ap()

    with tile.TileContext(nc) as tc:
        _trace_kernel(nc, tc, p1a_d, p1b_d, f32p_d,
                      wb_d, (w1s_d, w2s_d), (w1e_d, w2e_d), out_d)
    nc.compile()
    return nc


def _trace_kernel(nc, tc, p1a_d, p1b_d, f32p_d, wb_d, ws_d, we_d, out_d):
    import contextlib
    est = contextlib.ExitStack()
    with est:
        const = est.enter_context(tc.tile_pool(name="const", bufs=1))
        attn = est.enter_context(tc.tile_pool(name="attn", bufs=1))
        wres = est.enter_context(tc.tile_pool(name="wres", bufs=1))
        tring = est.enter_context(tc.tile_pool(name="tring", bufs=9))
        acts = est.enter_context(tc.tile_pool(name="acts", bufs=1))
        oplane = est.enter_context(tc.tile_pool(name="oplane", bufs=3))
        cols = est.enter_context(tc.tile_pool(name="cols", bufs=2))
        pmm = est.enter_context(tc.tile_pool(name="pmm", bufs=3, space="PSUM"))
        pffw = est.enter_context(tc.tile_pool(name="pffw", bufs=3, space="PSUM"))
        ptiny = est.enter_context(tc.tile_pool(name="ptiny", bufs=2, space="PSUM"))
        pat = ptiny

        def mm(out, lhsT, rhs, start, stop):
            nc.tensor.matmul(out, lhsT, rhs, start=start, stop=stop)

        # warm_r doubles as the all-ones operand (rank-1 terms, broadcasts)
        # and as the keepalive/warm-up moving tensor.
        warm_l = const.tile([128, 2], F16, tag="warm_l")
        nc.vector.memset(warm_l[:], 1.0)
        warm_r = const.tile([128, 512], F16, tag="warm_r")
        nc.vector.memset(warm_r[:], 1.0)
        ones_row = warm_r[0:1, :]
        ones2 = warm_r[:, 0:2]

        # keepalive: dense N=512 matmuls on memset operands.  The HAM clock
        # gate needs a ~3.4us window of HIGH busy fraction to open and stays
        # open only while the PE keeps a high duty cycle; these fill the
        # attention phase's PE-idle slots at ~215ns each.
        def fill(n=1):
            for _ in range(n):
                p = pmm.tile([2, 512], F32, tag="pmm", name="fill")
                nc.tensor.matmul(p[:], warm_l[:], warm_r[:], start=True, stop=True)

        # ---- DMA plan.  DMA packets round-robin across ACTIVE queues, so a
        # queue fronted by tiny-row packs crawls while others flow; every
        # critical transfer must ride coarse packets.  Queue assignment:
        #   sync:   ctxT|quesT|bf16consts b0, (same) b1, W1e, W2s, Wb
        #   scalar: f32p (tiny, first), ques|ctx b0, ques|ctx b1
        #   gpsimd: W1s, W2e (software DGE, measured ~265GB/s on 1MB)
        f32p = const.tile([128, NF32], F32, tag="f32p")
        nc.scalar.dma_start(out=f32p[:], in_=f32p_d[:])
        p1at, p1bt = {}, {}
        for b in (0, 1):
            p1at[b] = attn.tile([128, P1AW], F16, tag=f"p1a_{b}", name=f"p1a_{b}")
            nc.sync.dma_start(out=p1at[b][:], in_=p1a_d[b])
        for b in (0, 1):
            p1bt[b] = attn.tile([128, P1BW], F16, tag=f"p1b_{b}", name=f"p1b_{b}")
            nc.scalar.dma_start(out=p1bt[b][:], in_=p1b_d[b])
        bf16p = p1at[0][:, OFF_BF:OFF_BF + NBF16]

        colw = bf16p[:, OFF_COLW:OFF_COLW + 8]
        ident = bf16p[:, OFF_IDENT:OFF_IDENT + 128]
        wrow4 = bf16p[0:1, OFF_WROW4:OFF_WROW4 + 4]

        ctxT_sb = {b: [p1at[b][:, OFF_CT + hc * LC:OFF_CT + (hc + 1) * LC]
                       for hc in range(NHC)] for b in (0, 1)}
        quesT_sb = {b: [p1at[b][:, OFF_QT + hc * LQ:OFF_QT + (hc + 1) * LQ]
                        for hc in range(NHC)] for b in (0, 1)}
        ctx_sb = {b: [p1bt[b][:, OFF_CX + ic * H:OFF_CX + (ic + 1) * H]
                      for ic in range(NIC)] for b in (0, 1)}

        # bulk weights, first-use order (h1-s, h1-e, out-s, out-e, biaffine).
        wtiles = {}
        for lname, (w1_d, w2_d) in (("s", ws_d), ("e", we_d)):
            w1t = wres.tile([128, NDC, DFF], F16, tag=f"w1{lname}", name=f"w1{lname}")
            w2t = wres.tile([128, NFC, D], F16, tag=f"w2{lname}", name=f"w2{lname}")
            wtiles[lname] = (w1t, w2t)
        nc.gpsimd.dma_start(out=wtiles["s"][0][:], in_=ws_d[0][:])
        nc.sync.dma_start(out=wtiles["e"][0][:], in_=we_d[0][:])
        nc.sync.dma_start(out=wtiles["s"][1][:], in_=ws_d[1][:])
        nc.gpsimd.dma_start(out=wtiles["e"][1][:], in_=we_d[1][:])
        wbt_tile = wres.tile([128, NDC, C, D + 1], F16, tag="wb")
        nc.sync.dma_start(out=wbt_tile[:], in_=wb_d[:])

        def pre_h1s():
            """Pre-start the first three h1-s accumulation groups of batch 0
            with their two ctxT-chunk terms (deps: W1s + ctxT_b0 only).  Real
            dense work that fills the measured ~1.8us PE hole while batch
            0's serial attention tail (aT evac -> c2q -> xT) produces the
            remaining xT chunks.  Groups stay open on the three pmm bufs
            until ffw(0) finishes them."""
            w1t = wtiles["s"][0]
            x67 = A[0]["xT67"]
            xTp = {0: ctxT_sb[0][0], 1: ctxT_sb[0][1], 6: x67[0][:], 7: x67[1][:]}
            pres = []
            for fc in range(NFC):
                if fc < 3:
                    p = pmm.tile([128, LC], F32, tag="pmm", name=f"preh1s{fc}")
                else:
                    p = pffw.tile([128, LC], F32, tag="pf", name=f"preh1s{fc}")
                for k, dc in enumerate((0, 1, 6, 7)):
                    mm(p[:], w1t[:, dc, fc * 128:(fc + 1) * 128],
                       xTp[dc], start=(k == 0), stop=False)
                pres.append(p)
            return pres

        A = {0: {}, 1: {}}

        def attn_stage1(b):
            """(ctx*w3)^T + stacked rank-2 rows: q2ones = [ones; ques@w2],
            c1ones = [ctx@w1; ones] (both broadcast sim terms become ONE
            accumulation matmul into psim)."""
            st = A[b]
            st["ctxw3T"] = []
            for hc in range(NHC):
                t_ = attn.tile([128, LC], F16, tag=f"ctxw3T{hc}_{b}")
                nc.vector.tensor_scalar_mul(
                    t_[:], ctxT_sb[b][hc],
                    f32p[:, COL_W3 + hc:COL_W3 + hc + 1])
                st["ctxw3T"].append(t_)
            o10 = bf16p[0:1, OFF_Z:OFF_Z + 2]
            o01 = bf16p[0:1, OFF_Z + 1:OFF_Z + 3]
            p_q2r = pat.tile([2, LQ], F32, tag="pt", name=f"pq2r_{b}")
            for hc in range(NHC):
                mm(p_q2r[:], colw[:, 4 + 2 * hc:6 + 2 * hc], quesT_sb[b][hc],
                   start=(hc == 0), stop=False)
            mm(p_q2r[:], o10, ones_row[:, 0:LQ], start=False, stop=True)
            st["q2ones"] = cols.tile([2, LQ], F16, tag="q2row", name=f"q2row_{b}")
            nc.scalar.activation(st["q2ones"][:], p_q2r[:], AF.Copy)
            p_c1r = pffw.tile([2, LC], F32, tag="pf", name=f"pc1r_{b}")
            for hc in range(NHC):
                mm(p_c1r[:], colw[:, 2 * hc:2 * hc + 2], ctxT_sb[b][hc],
                   start=(hc == 0), stop=False)
            mm(p_c1r[:], o01, ones_row[:, 0:LC], start=False, stop=True)
            st["c1ones"] = cols.tile([2, LC], F16, tag="c1row", name=f"c1row_{b}")
            nc.scalar.activation(st["c1ones"][:], p_c1r[:], AF.Copy)

        def attn_stage2(b, fil=None):
            """sim + softmax-over-j, per 128-row i chunk."""
            st = A[b]
            st["ucols"] = attn.tile([128, NIC + 2], F16, tag=f"ucols_{b}",
                                    name=f"ucols_{b}")
            st["a_n"] = []
            for ic in range(NIC):
                icsl = slice(ic * 128, (ic + 1) * 128)
                p_sim = pmm.tile([128, LQ], F32, tag="pmm", name=f"psim_{b}{ic}")
                for hc in range(NHC):
                    mm(p_sim[:], st["ctxw3T"][hc][:, icsl], quesT_sb[b][hc],
                       start=(hc == 0), stop=False)
                mm(p_sim[:], st["c1ones"][:, icsl], st["q2ones"][:],
                   start=False, stop=True)
                a_un = attn.tile([128, LQ], F32, tag=f"aun{ic}_{b}")
                nc.scalar.activation(a_un[:], p_sim[:], AF.Exp)
                ssum = cols.tile([128, 1], F32, tag="ssum", name=f"ssum_{b}{ic}")
                nc.vector.reduce_sum(out=ssum[:], in_=a_un[:], axis=AX.X)
                srec = cols.tile([128, 1], F32, tag="srec", name=f"srec_{b}{ic}")
                nc.vector.reciprocal(srec[:], ssum[:])
                nc.vector.reduce_max(out=st["ucols"][:, ic:ic + 1], in_=a_un[:],
                                     axis=AX.X)
                t_ = attn.tile([128, LQ], F16, tag=f"an{ic}_{b}")
                nc.vector.tensor_scalar_mul(t_[:], a_un[:], srec[:])
                st["a_n"].append(t_)
                if fil:
                    fil(2)

        def attn_stage3(b, fil=None):
            """a^T transposes, softmax-over-i weights, q2c column."""
            st = A[b]
            st["aT"] = attn.tile([LQ, LC], F16, tag=f"aT_{b}", name=f"aT_{b}")
            for ic in range(NIC):
                p = pffw.tile([LQ, 128], F16, tag="pf", name=f"paT_{b}{ic}")
                nc.tensor.transpose(p[:], st["a_n"][ic][:], ident)
                nc.scalar.activation(st["aT"][:, ic * 128:(ic + 1) * 128], p[:],
                                     AF.Copy)
            ucol1 = cols.tile([128, 1], F16, tag="ucol1", name=f"ucol1_{b}")
            with nc.allow_low_precision(reason="4-term bf16 softmax-denominator partial sum"):
                nc.vector.reduce_sum(out=ucol1[:], in_=st["ucols"][:, 0:NIC],
                                     axis=AX.X)
            p_den = pat.tile([1, 2], F32, tag="pt", name=f"pden_{b}")
            mm(p_den[:], ucol1[:], ones2, start=True, stop=True)
            inv2f = cols.tile([1, 2], F32, tag="inv2f", name=f"inv2f_{b}")
            nc.vector.reciprocal(inv2f[:], p_den[:])
            inv2 = cols.tile([1, 2], F16, tag="inv2", name=f"inv2_{b}")
            nc.scalar.activation(inv2[:], inv2f[:], AF.Copy)
            p_bc = pat.tile([128, 2], F32, tag="pt", name=f"pbc_{b}")
            mm(p_bc[:], ones_row[:, 0:128], inv2[:], start=True, stop=True)
            invb = cols.tile([128, 1], F32, tag="invb", name=f"invb_{b}")
            nc.scalar.activation(invb[:], p_bc[:, 0:1], AF.Copy)
            if fil:
                fil(2)
            st["q2cc"] = []
            for hs in range(NHC):
                p_q2c = pat.tile([128, 2], F32, tag="pt", name=f"pq2c_{b}{hs}")
                for ic in range(NIC):
                    mm(p_q2c[:], ctx_sb[b][ic][:, hs * 128:(hs + 1) * 128],
                       st["ucols"][:, ic:ic + 2], start=(ic == 0),
                       stop=(ic == NIC - 1))
                t_ = cols.tile([128, 1], F32, tag=f"q2cc{hs}", name=f"q2cc_{b}{hs}")
                nc.vector.tensor_mul(t_[:], p_q2c[:, 0:1], invb[:])
                st["q2cc"].append(t_)
            st["xT67"] = []
            for hc in range(NHC):
                t_ = acts.tile([128, LC], F16, tag=f"xT{6 + hc}_{b}")
                nc.vector.tensor_scalar_mul(t_[:], ctxT_sb[b][hc],
                                            st["q2cc"][hc][:])
                st["xT67"].append(t_)

        def attn_stage4(b):
            """x^T assembly: 0-1 ctx^T, 2-3 c2q^T, 4-5 (ctx*c2q)^T,
            6-7 (ctx*q2c)^T.  6-7 are emitted first (deps ready earliest)."""
            st = A[b]
            xT = [None] * 8
            xT[0], xT[1] = ctxT_sb[b][0], ctxT_sb[b][1]
            xT[6], xT[7] = st["xT67"]
            for hs in range(NHC):
                p_c2q = pffw.tile([128, LC], F32, tag="pf", name=f"pc2q_{b}{hs}")
                mm(p_c2q[:],
                   p1bt[b][0:LQ, OFF_Q + hs * 128:OFF_Q + (hs + 1) * 128],
                   st["aT"][:], start=True, stop=True)
                t_ = acts.tile([128, LC], F16, tag=f"xT{2 + hs}_{b}")
                if hs == 0:
                    nc.scalar.activation(t_[:], p_c2q[:], AF.Copy)
                else:
                    nc.vector.tensor_copy(t_[:], p_c2q[:])
                xT[2 + hs] = t_
            for hc in range(NHC):
                t_ = acts.tile([128, LC], F16, tag=f"xT{4 + hc}_{b}")
                nc.vector.tensor_mul(t_[:], ctxT_sb[b][hc], xT[2 + hc][:])
                xT[4 + hc] = t_
            st["xT"] = xT
            return xT

        # first-chunk accumulation order: late-produced xT chunks last.
        DC_FIRST = [0, 1, 6, 7, 2, 3, 4, 5]

        def ffw(b, xT, hook=None, pre=None):
            # psum groups alternate between the pffw and pmm pools (6 bufs
            # of WAR slack instead of 3) and evacuations alternate between
            # the vector and scalar engines, so batch-1 attention work
            # hooked into this stream cannot stall the PE on a delayed
            # evacuation.
            h1 = {}
            gidx = 0
            for si, (lname, colb1) in enumerate((("s", COL_B1S), ("e", COL_B1E))):
                w1t = wtiles[lname][0]
                h1[lname] = []
                if pre is not None and lname == "s":
                    # continue the pre-started groups dc-MAJOR: all four
                    # groups consume xT[2,3] first (8 MMs of runway) before
                    # any needs the later-produced xT[4,5].
                    for dc in (2, 3, 4, 5):
                        for fc in range(NFC):
                            mm(pre[fc][:], w1t[:, dc, fc * 128:(fc + 1) * 128],
                               xT[dc], start=False, stop=(dc == 5))
                    for fc in range(NFC):
                        t_ = acts.tile([128, LC], F16, tag=f"h1{lname}{fc}",
                                       name=f"h1{lname}{fc}_{b}")
                        if gidx % 2 == 0:
                            nc.vector.tensor_scalar(
                                out=t_[:], in0=pre[fc][:],
                                scalar1=f32p[:, colb1 + fc:colb1 + fc + 1],
                                scalar2=0.0, op0=ALU.add, op1=ALU.max)
                        else:
                            nc.scalar.activation(
                                t_[:], pre[fc][:], AF.Relu,
                                bias=f32p[:, colb1 + fc:colb1 + fc + 1], scale=1.0)
                        h1[lname].append(t_)
                        gidx += 1
                    if hook:
                        hook(si)
                    continue
                for fc in range(NFC):
                    pool = pffw if gidx % 2 == 0 else pmm
                    tagp = "pf" if gidx % 2 == 0 else "pmm"
                    p = pool.tile([128, LC], F32, tag=tagp, name=f"ph1{lname}_{b}{fc}")
                    order = DC_FIRST if (si == 0) else range(NDC)
                    for k, dc in enumerate(order):
                        mm(p[:], w1t[:, dc, fc * 128:(fc + 1) * 128], xT[dc],
                           start=(k == 0), stop=(k == NDC - 1))
                    t_ = acts.tile([128, LC], F16, tag=f"h1{lname}{fc}",
                                   name=f"h1{lname}{fc}_{b}")
                    if gidx % 2 == 0:
                        nc.vector.tensor_scalar(
                            out=t_[:], in0=p[:],
                            scalar1=f32p[:, colb1 + fc:colb1 + fc + 1],
                            scalar2=0.0, op0=ALU.add, op1=ALU.max)
                    else:
                        nc.scalar.activation(
                            t_[:], p[:], AF.Relu,
                            bias=f32p[:, colb1 + fc:colb1 + fc + 1], scale=1.0)
                    h1[lname].append(t_)
                    gidx += 1
                if hook:
                    hook(si)
            sT, eT = [], []
            for si, (lname, colb2, dst) in enumerate(
                    (("s", COL_B2S, sT), ("e", COL_B2E, eT))):
                w2t = wtiles[lname][1]
                for dc in range(NDC):
                    pool = pffw if gidx % 2 == 0 else pmm
                    tagp = "pf" if gidx % 2 == 0 else "pmm"
                    p = pool.tile([128, LC], F32, tag=tagp, name=f"po{lname}_{b}{dc}")
                    for fc in range(NFC):
                        mm(p[:], w2t[:, fc, dc * 128:(dc + 1) * 128],
                           h1[lname][fc][:], start=(fc == 0), stop=(fc == NFC - 1))
                    t_ = acts.tile([128, LC], F16, tag=f"{lname}T{dc}", bufs=2,
                                   name=f"{lname}T{dc}_{b}")
                    if gidx % 2 == 0:
                        nc.vector.tensor_scalar_add(
                            t_[:], p[:], f32p[:, colb2 + dc:colb2 + dc + 1])
                    else:
                        nc.scalar.activation(
                            t_[:], p[:], AF.Identity,
                            bias=f32p[:, colb2 + dc:colb2 + dc + 1],
                            scale=1.0)
                    dst.append(t_)
                    gidx += 1
                if hook:
                    hook(2 + si)
            return sT, eT

        def biaffine(b, sT, eT):
            # t1 rows for all three labels in one group:
            # t1[c, x] = sum_i start^T[i, x] * Wb[i, c, D]  + Wb[D, c, D]
            p_t14 = pffw.tile([4, LC], F32, tag="pf", name=f"pt14_{b}")
            for ic in range(NDC):
                mm(p_t14[:], bf16p[:, OFF_UPACK + ic * 4:OFF_UPACK + ic * 4 + 4],
                   sT[ic][:], start=(ic == 0), stop=False)
            mm(p_t14[:], wrow4, ones_row, start=False, stop=True)
            t14 = cols.tile([4, LC], F16, tag="t14", name=f"t14_{b}")
            nc.scalar.activation(t14[:], p_t14[:], AF.Copy)
            t1cols = []
            for xc in range(NIC):
                p = ptiny.tile([128, 4], F16, tag="pt", name=f"pt1c_{b}{xc}")
                nc.tensor.transpose(p[:], t14[:, xc * 128:(xc + 1) * 128],
                                    ident[0:4, 0:4])
                tsb = cols.tile([128, 4], F32, tag=f"t1c{xc}", name=f"t1c{xc}_{b}")
                nc.vector.tensor_copy(tsb[:], p[:])
                t1cols.append(tsb)

            for c in range(C):
                # t_c^T[j, x] = sum_i Wb[i,c,j] * start^T[i, x]  (+ v_c[j])
                tt = []
                for jc in range(NJC):
                    p = pmm.tile([128, LC], F32, tag="pmm", name=f"pt_{b}{c}{jc}")
                    for ic in range(NDC):
                        mm(p[:], wbt_tile[:, ic, c, jc * 128:(jc + 1) * 128],
                           sT[ic][:], start=(ic == 0), stop=(ic == NDC - 1))
                    t_ = tring.tile([128, LC], F16, tag="t", name=f"t_{b}{c}{jc}")
                    nc.vector.tensor_scalar_add(
                        t_[:], p[:],
                        f32p[:, COL_VC + c * NJC + jc:COL_VC + c * NJC + jc + 1])
                    tt.append(t_)

                # score_c[x, y] = sum_j t_c^T[j, x] * end^T[j, y] + t1_c[x],
                # accumulated into a [128, 4, LC] bf16 plane group, DMA'd out
                # as one [LC, LC] label plane on the sync hardware queue.
                planes = oplane.tile([128, NIC, LC], F16, tag="opl",
                                     name=f"opl_{b}{c}")
                last = (b == NB - 1 and c == C - 1)
                for xc in range(NIC):
                    if last and xc == NIC - 1:
                        # final plane chunk in two halves: half A's evac+DMA
                        # overlap half B's matmuls, halving the tail chain.
                        for h in range(2):
                            ysl = slice(h * 256, (h + 1) * 256)
                            ph = pmm.tile([128, 256], F32, tag="pmm",
                                          name=f"ps_{b}{c}{xc}{h}")
                            for jc in range(NJC):
                                mm(ph[:], tt[jc][:, xc * 128:(xc + 1) * 128],
                                   eT[jc][:, ysl], start=(jc == 0),
                                   stop=(jc == NJC - 1))
                            if h == 0:
                                nc.scalar.activation(
                                    planes[:, xc, 0:256], ph[:], AF.Identity,
                                    bias=t1cols[xc][:, c:c + 1], scale=1.0)
                            else:
                                nc.vector.tensor_scalar_add(
                                    planes[:, xc, 256:512], ph[:],
                                    t1cols[xc][:, c:c + 1])
                            nc.sync.dma_start(
                                out=out_d[b, c, :, xc, ysl],
                                in_=planes[:, xc, ysl])
                        continue
                    p = pmm.tile([128, LC], F32, tag="pmm", name=f"ps_{b}{c}{xc}")
                    for jc in range(NJC):
                        mm(p[:], tt[jc][:, xc * 128:(xc + 1) * 128], eT[jc][:],
                           start=(jc == 0), stop=(jc == NJC - 1))
                    if last and xc % 2 == 1:
                        nc.vector.tensor_scalar_add(planes[:, xc, :], p[:],
                                                    t1cols[xc][:, c:c + 1])
                    else:
                        nc.scalar.activation(planes[:, xc, :], p[:], AF.Identity,
                                             bias=t1cols[xc][:, c:c + 1], scale=1.0)
                    if last:
                        nc.sync.dma_start(out=out_d[b, c, :, xc, :],
                                          in_=planes[:, xc, :])
                if not last:
                    nc.sync.dma_start(out=out_d[b, c], in_=planes[:])

        # ---- schedule ----
        # warm-up burst while the input DMAs land, then A0 solo (keepalive
        # fillers hold the HAM gate open), then FFW0 with A1's stages hooked
        # between its blocks, then FFW1 / B0 / B1 back-to-back dense.
        fill(8)
        attn_stage1(0)
        attn_stage2(0)
        attn_stage3(0)
        pres = pre_h1s()
        fill(4)
        xT0 = attn_stage4(0)

        def hook(si):
            if si == 0:
                attn_stage1(1)
            elif si == 1:
                attn_stage2(1)
            elif si == 2:
                attn_stage3(1)
            else:
                attn_stage4(1)

        se0 = ffw(0, xT0, hook=hook, pre=pres)
        se1 = ffw(1, A[1]["xT"])
        biaffine(0, *se0)
        biaffine(1, *se1)


_PROGRAM_CACHE = {}


def _get_program():
    if "nc" not in _PROGRAM_CACHE:
        _PROGRAM_CACHE["nc"] = _build_program()
    return _PROGRAM_CACHE["nc"]


def _pack_host_inputs(w_sim, W1s, b1s, W2s, b2s, W1e, b1e, W2e, b2e, Wb):
    """Build the shared (replicated) input arrays from the raw weights."""
    import ml_dtypes
    f32, f16 = np.float32, ml_dtypes.bfloat16
    w1, w2, w3 = [np.asarray(w_sim[k * H:(k + 1) * H], f32) for k in range(3)]

    f32p = np.zeros((128, NF32), f32)
    for hc in range(NHC):
        f32p[:, COL_W3 + hc] = w3[hc * 128:(hc + 1) * 128]
    for fc in range(NFC):
        f32p[:, COL_B1S + fc] = b1s[fc * 128:(fc + 1) * 128]
        f32p[:, COL_B1E + fc] = b1e[fc * 128:(fc + 1) * 128]
    for dc in range(NDC):
        f32p[:, COL_B2S + dc] = b2s[dc * 128:(dc + 1) * 128]
        f32p[:, COL_B2E + dc] = b2e[dc * 128:(dc + 1) * 128]
    for c in range(C):
        for jc in range(NJC):
            f32p[:, COL_VC + c * NJC + jc] = Wb[D, c, jc * 128:(jc + 1) * 128]

    bf16p = np.zeros((128, NBF16), f32)
    for hc in range(NHC):
        bf16p[:, OFF_COLW + 2 * hc] = w1[hc * 128:(hc + 1) * 128]
        bf16p[:, OFF_COLW + 5 + 2 * hc] = w2[hc * 128:(hc + 1) * 128]
    bf16p[:, OFF_IDENT:OFF_IDENT + 128] = np.eye(128, dtype=f32)
    for dc in range(NDC):
        for c in range(C):
            bf16p[:, OFF_UPACK + dc * 4 + c] = Wb[dc * 128:(dc + 1) * 128, c, D]
    bf16p[:, OFF_WROW4:OFF_WROW4 + C] = Wb[D, :, D][None, :]
    bf16p[:, OFF_Z] = 1.0
    bf16p[:, OFF_Z + 2] = 1.0

    def pmaj(a, nchunk):
        # [nchunk*128, F...] -> [128, nchunk, F...] (SBUF-tile layout)
        return np.ascontiguousarray(
            a.reshape((nchunk, 128) + a.shape[1:]).swapaxes(0, 1))

    return {
        "f32pack": f32p,
        "_bf16pack": bf16p.astype(f16),
        "wb": pmaj(Wb[:D].astype(f16), NDC),
        "W1s": pmaj(W1s.astype(f16), NDC),
        "W2s": pmaj(W2s.astype(f16), NFC),
        "W1e": pmaj(W1e.astype(f16), NDC),
        "W2e": pmaj(W2e.astype(f16), NFC),
    }


def kernel(ctx_emb, ques_emb, w_sim, W1s, b1s, W2s, b2s, W1e, b1e, W2e, b2e, Wb,
           _trace=False, _tmpdir=None):
    from concourse.bass_utils import run_bass_kernel_spmd
    import ml_dtypes

    # accept jax/np arrays of any layout
    (ctx_emb, ques_emb, w_sim, W1s, b1s, W2s, b2s, W1e, b1e, W2e, b2e, Wb) = (
        np.asarray(a, dtype=np.float32)
        for a in (ctx_emb, ques_emb, w_sim, W1s, b1s, W2s, b2s, W1e, b1e, W2e,
                  b2e, Wb))

    nc = _get_program()
    shared = _pack_host_inputs(w_sim, W1s, b1s, W2s, b2s, W1e, b1e, W2e, b2e, Wb)
    f16 = ml_dtypes.bfloat16
    ctx16 = ctx_emb.astype(f16)
    ques16 = ques_emb.astype(f16)
    # per-batch input packs: p1a = ctxT | quesT | bf16 consts;
    # p1b = ques (rows 0:64) | ctx
    bf16pack = shared.pop("_bf16pack")
    p1a = np.zeros((B, 128, P1AW), f16)
    p1a[:, :, OFF_CT:OFF_CT + 1024] = (
        ctx16.transpose(0, 2, 1).reshape(B, NHC, 128, LC)
        .swapaxes(1, 2).reshape(B, 128, NHC * LC))
    p1a[:, :, OFF_QT:OFF_QT + 128] = (
        ques16.transpose(0, 2, 1).reshape(B, NHC, 128, LQ)
        .swapaxes(1, 2).reshape(B, 128, NHC * LQ))
    p1a[:, :, OFF_BF:OFF_BF + NBF16] = bf16pack[None]
    p1b = np.zeros((B, 128, P1BW), f16)
    p1b[:, 0:LQ, OFF_Q:OFF_Q + H] = ques16
    p1b[:, :, OFF_CX:OFF_CX + 1024] = (
        ctx16.reshape(B, NIC, 128, H).swapaxes(1, 2).reshape(B, 128, NIC * H))
    in_maps = []
    for core in range(N_CORES):
        sl = slice(core * NB, (core + 1) * NB)
        in_maps.append({"p1a": np.ascontiguousarray(p1a[sl]),
                        "p1b": np.ascontiguousarray(p1b[sl]), **shared})

    kw = {}
    if _trace:
        kw = {"trace": True, "tmpdir": _tmpdir}
    res = run_bass_kernel_spmd(nc, in_maps, list(range(N_CORES)), **kw)
    # device layout is [NB, C, 128, NIC, LC] bf16 (p-major planes);
    # upcast + unpermute to [B, LC, LC, C] on the host.
    outs = []
    for i in range(N_CORES):
        o = np.asarray(res.results[i]["out"])  # [NB, C, 128, NIC, LC]
        o = o.astype(np.float32).transpose(0, 3, 2, 4, 1)  # [NB, NIC, 128, LC, C]
        outs.append(o.reshape(NB, LC, LC, C))
    out = np.ascontiguousarray(np.concatenate(outs, axis=0))
    if _trace:
        return out, res
    return out


# revision 36
# speedup vs baseline: 1.0097x; 1.0097x over previous
"""Trainium2 Bass kernel for nn_BiaffineNER (BiDAF attention + FFW + biaffine scorer).

Contract: kernel(**inputs) takes the FULL unsharded inputs (numpy) and returns
the FULL [16, 512, 512, 3] float32 output. Internally shards data-parallel over
the batch axis across 8 NeuronCores (2 batch elements per core), runs one SPMD
Bass/Tile program on all cores, and concatenates the per-core outputs.

Math per batch element b (LC=512, LQ=64, H=256, D=4H=1024, DFF=512, C=3):
  sim  = (ctx@w1)[:,None] + (ques@w2)[None,:] + (ctx*w3)@ques.T      [LC,LQ]
  a    = softmax_j(sim); c2q = a @ ques                              [LC,H]
  bwt  = softmax_i(max_j sim); q2c = bwt @ ctx                       [H]
  x    = [ctx, c2q, ctx*c2q, ctx*q2c]                                [LC,D]
  start= relu(x@W1s+b1s)@W2s+b2s ; end likewise                      [LC,D]
  out[x,y,c] = [start,1][x] . Wb[:,c,:] . [end,1][y]                 [LC,LC,C]

Design notes:
- Activations kept transposed on-chip ([feature-part, token-free]) so the
  contraction dim always sits on SBUF partitions; ctx^T/ques^T come in
  host-pretransposed.
- All matmul operands are bfloat16: the PE runs bf16 at 1 cycle/row (full
  rate; fp32r pays a serialized 4-byte LDWEIGHTS ~176ns/matmul, and IEEE fp16
  measures 2 cycles/row on real HW).  End-to-end rel err ~6e-3 (tolerance
  2e-2); accumulation stays fp32 in PSUM.
- All weights (FFW + the 12.6MB Wb) are loaded once in bf16 and stay resident
  in SBUF for both batch elements: HBM read drops ~44MB -> ~12MB per core.
- dma_start issue cost is ~0.6-0.75us of engine time each, so DMAs are merged
  aggressively (host arrays pre-packed p-major so merged DMAs stay contiguous):
  2 constant packs, 1 DMA per FFW weight matrix, 1 DMA for all of Wb, 4 DMAs
  per batch of inputs, one output DMA per (batch, label) plane group.
- DMA queue plan: each hw queue (scalar/Activation, sync/SP) carries one
  batch's critical inputs FIRST; all 10.3MB of bulk weights then ride the
  sync queue alone, ordered by first use (the Activation queue only sustains
  ~90GB/s while sync's is active; sync alone does ~290GB/s).  Output planes
  also leave via sync.  gpsimd's software queue (~43GB/s) carries nothing.
- The output leaves as bf16 [C, LC, LC] planes (host transposes/upcasts for
  free), so the kernel tail is one 512KB DMA, not a whole batch element.
- Softmaxes skip max-subtraction (|sim| < ~8 for this data distribution), which
  turns the partition-axis softmax over i into tiny matmul reductions.
- The two batch elements' attention front-ends are instruction-interleaved
  (independent dependency chains hide each other's latency), then
  F0 F1 B0 B1, with a ~3us dependency-free warm-up matmul burst up front so
  the PE HAM clock gate opens before the first dense phase.
"""

import sys

if "/opt/trn_rl_repo" not in sys.path and "/root/.axon_site/_ro/trn_rl_repo" not in sys.path:
    sys.path.insert(0, "/opt/trn_rl_repo")

import numpy as np

import concourse.bass as bass
import concourse.tile as tile
from concourse import bacc, mybir

F32 = mybir.dt.float32
F16 = mybir.dt.bfloat16
AF = mybir.ActivationFunctionType
ALU = mybir.AluOpType
AX = mybir.AxisListType

N_CORES = 8
B, LC, LQ, H = 16, 512, 64, 256
NB = B // N_CORES          # batch elements per core
D, DFF, C = 4 * H, 512, 3
NIC = LC // 128            # 4  i/x chunks
NHC = H // 128             # 2  h chunks
NDC = D // 128             # 8  d chunks
NFC = DFF // 128           # 4  f chunks
NJC = 8                    # j chunks (first 1024 of 1025)

# f32 constant pack column layout [128, 54]
COL_W3 = 0                 # 2 cols (w3 chunks, per-partition scalars)
COL_B1S, COL_B2S = 2, 6    # 4 + 8
COL_B1E, COL_B2E = 14, 18  # 4 + 8
COL_VC = 26                # 24 cols: vcols[c*NJC + jc]
NF32 = 50

# bf16 constant pack column layout [128, 687]
OFF_ONES = 0               # 512 cols of 1.0 (ones_row row 0; ones2 any 2 cols)
OFF_COLW = 512             # 8 cols: [w1_0, 0, w1_1, 0, 0, w2_0, 0, w2_1]
OFF_IDENT = 520            # 128 cols identity
OFF_UPACK = 648            # 32 cols: upack[dc*4 + c] = Wb[dc*128+p, c, D]
OFF_WROW4 = 680            # 4 cols: Wb[D, c, D] broadcast down partitions
OFF_Z = 684                # 3 cols [1, 0, 1]: [z:z+2]=[1,0], [z+1:z+3]=[0,1]
NBF16 = 687


def _build_program():
    nc = bacc.Bacc("TRN2", target_bir_lowering=False, debug=False,
                   num_devices=N_CORES)

    ctx_d = nc.dram_tensor("ctx", [NB, 128, NIC, H], F16, kind="ExternalInput").ap()
    ques_d = nc.dram_tensor("ques", [NB, LQ, H], F16, kind="ExternalInput").ap()
    ctxT_d = nc.dram_tensor("ctxT", [NB, 128, NHC, LC], F16, kind="ExternalInput").ap()
    quesT_d = nc.dram_tensor("quesT", [NB, 128, NHC, LQ], F16, kind="ExternalInput").ap()
    f32p_d = nc.dram_tensor("f32pack", [128, NF32], F32, kind="ExternalInput").ap()
    bf16p_d = nc.dram_tensor("bf16pack", [128, NBF16], F16, kind="ExternalInput").ap()
    wb_d = nc.dram_tensor("wb", [128, NDC, C, D + 1], F16, kind="ExternalInput").ap()
    w1s_d = nc.dram_tensor("W1s", [128, NDC, DFF], F16, kind="ExternalInput").ap()
    w2s_d = nc.dram_tensor("W2s", [128, NFC, D], F16, kind="ExternalInput").ap()
    w1e_d = nc.dram_tensor("W1e", [128, NDC, DFF], F16, kind="ExternalInput").ap()
    w2e_d = nc.dram_tensor("W2e", [128, NFC, D], F16, kind="ExternalInput").ap()
    out_d = nc.dram_tensor("out", [NB, C, 128, NIC, LC], F16, kind="ExternalOutput").ap()

    with tile.TileContext(nc) as tc:
        _trace_kernel(nc, tc, ctx_d, ques_d, ctxT_d, quesT_d, f32p_d, bf16p_d,
                      wb_d, (w1s_d, w2s_d), (w1e_d, w2e_d), out_d)
    nc.compile()
    return nc


def _trace_kernel(nc, tc, ctx_d, ques_d, ctxT_d, quesT_d, f32p_d, bf16p_d,
                  wb_d, ws_d, we_d, out_d):
    import contextlib
    est = contextlib.ExitStack()
    with est:
        const = est.enter_context(tc.tile_pool(name="const", bufs=1))
        attn = est.enter_context(tc.tile_pool(name="attn", bufs=1))
        wres = est.enter_context(tc.tile_pool(name="wres", bufs=1))
        tring = est.enter_context(tc.tile_pool(name="tring", bufs=9))
        acts = est.enter_context(tc.tile_pool(name="acts", bufs=1))
        oplane = est.enter_context(tc.tile_pool(name="oplane", bufs=3))
        cols = est.enter_context(tc.tile_pool(name="cols", bufs=2))
        pmm = est.enter_context(tc.tile_pool(name="pmm", bufs=3, space="PSUM"))
        pffw = est.enter_context(tc.tile_pool(name="pffw", bufs=3, space="PSUM"))
        ptiny = est.enter_context(tc.tile_pool(name="ptiny", bufs=2, space="PSUM"))
        pat = ptiny

        def mm(out, lhsT, rhs, start, stop):
            nc.tensor.matmul(out, lhsT, rhs, start=start, stop=stop)

        # HAM warm-up: the clock gate needs a ~3.4us window of HIGH busy
        # FRACTION to open -- tiny [2,2] matmuls never register (measured: the
        # gate stayed at 4/8 until ~30us, running all of attention and the
        # first ~8us of FFW at 1.2GHz).  A dense burst of 512-row matmuls on
        # memset operands opens it at ~10us, inside the input-DMA wait.
        warm_l = const.tile([128, 2], F16, tag="warm_l")
        nc.vector.memset(warm_l[:], 1.0)
        warm_r = const.tile([128, 512], F16, tag="warm_r")
        nc.vector.memset(warm_r[:], 1.0)
        p_warm = pmm.tile([128, 512], F32, tag="pmm")
        for wi in range(12):
            nc.tensor.matmul(p_warm[0:2, :], warm_l[:], warm_r[:],
                             start=(wi == 0), stop=(wi == 11))

        # ---- DMA plan: each hw queue carries one batch's critical inputs
        # FIRST (plus one const pack), then its share of the bulk weights.
        # Criticals on both queues drain in parallel at full HBM rate before
        # any bulk weight competes for bandwidth.
        f32p = const.tile([128, NF32], F32, tag="f32p")
        nc.scalar.dma_start(out=f32p[:], in_=f32p_d[:])
        bf16p = const.tile([128, NBF16], F16, tag="bf16p")
        nc.sync.dma_start(out=bf16p[:], in_=bf16p_d[:])

        colw = bf16p[:, OFF_COLW:OFF_COLW + 8]
        ident = bf16p[:, OFF_IDENT:OFF_IDENT + 128]
        ones_row = bf16p[0:1, OFF_ONES:OFF_ONES + 512]
        ones2 = bf16p[:, OFF_ONES:OFF_ONES + 2]
        wrow4 = bf16p[0:1, OFF_WROW4:OFF_WROW4 + 4]

        quesT_sb, ctxT_sb, ques_sb, ctx_sb = {}, {}, {}, {}
        for b, eng in ((0, nc.scalar), (1, nc.sync)):
            t_ = attn.tile([128, NHC, LQ], F16, tag=f"quesT_{b}", name=f"quesT_{b}")
            eng.dma_start(out=t_[:], in_=quesT_d[b])
            quesT_sb[b] = [t_[:, hc, :] for hc in range(NHC)]
            t_ = attn.tile([128, NHC, LC], F16, tag=f"ctxT_{b}", name=f"ctxT_{b}")
            eng.dma_start(out=t_[:], in_=ctxT_d[b])
            ctxT_sb[b] = [t_[:, hc, :] for hc in range(NHC)]
            q_ = attn.tile([LQ, H], F16, tag=f"ques_{b}", name=f"ques_{b}")
            eng.dma_start(out=q_[:], in_=ques_d[b, :, :])
            ques_sb[b] = q_
            t_ = attn.tile([128, NIC, H], F16, tag=f"ctx_{b}", name=f"ctx_{b}")
            eng.dma_start(out=t_[:], in_=ctx_d[b])
            ctx_sb[b] = [t_[:, ic, :] for ic in range(NIC)]

        # bulk weights, after the criticals, ALL on the sync queue ordered by
        # first use (w1s ~20us ... Wb ~80us).  The scalar (Activation) hw
        # queue only gets ~90GB/s when sync's queue is active, so bulk on it
        # arrives late; sync alone moves 10.3MB well before each deadline.
        wtiles = {}
        for lname, (w1_d, w2_d) in (("s", ws_d), ("e", we_d)):
            w1t = wres.tile([128, NDC, DFF], F16, tag=f"w1{lname}", name=f"w1{lname}")
            nc.sync.dma_start(out=w1t[:], in_=w1_d[:])
            w2t = wres.tile([128, NFC, D], F16, tag=f"w2{lname}", name=f"w2{lname}")
            nc.sync.dma_start(out=w2t[:], in_=w2_d[:])
            wtiles[lname] = (w1t, w2t)
        wbt_tile = wres.tile([128, NDC, C, D + 1], F16, tag="wb")
        nc.sync.dma_start(out=wbt_tile[:], in_=wb_d[:])

        def attention_pair():
            """Both batches' attention, instruction-interleaved stage by stage.
            Returns {b: xT chunk list} (8 tiles [128, LC] bf16 each)."""
            BS = (0, 1)
            quesT, ctxT = quesT_sb, ctxT_sb

            # (ctx*w3)^T
            ctxw3T = {b: [] for b in BS}
            for b in BS:
                for hc in range(NHC):
                    t_ = attn.tile([128, LC], F16, tag=f"ctxw3T{hc}_{b}")
                    nc.vector.tensor_scalar_mul(
                        t_[:], ctxT[b][hc],
                        f32p[:, COL_W3 + hc:COL_W3 + hc + 1])
                    ctxw3T[b].append(t_)

            # Stacked rank-2 tiles, built wholly in PSUM via zero-padded
            # weight columns + a rank-1 ones term: q2ones = [ones; ques@w2],
            # c1ones = [ctx@w1; ones].  Both broadcast terms of sim are then
            # ONE matmul: c1ones[:,isl].T @ q2ones = c1[i]*1 + 1*q2[j].
            o10 = bf16p[0:1, OFF_Z:OFF_Z + 2]
            o01 = bf16p[0:1, OFF_Z + 1:OFF_Z + 3]
            q2ones, c1ones = {}, {}
            for b in BS:
                p_q2r = pat.tile([2, LQ], F32, tag="pt", name=f"pq2r_{b}")
                for hc in range(NHC):
                    mm(p_q2r[:], colw[:, 4 + 2 * hc:6 + 2 * hc], quesT[b][hc],
                       start=(hc == 0), stop=False)
                mm(p_q2r[:], o10, ones_row[:, 0:LQ], start=False, stop=True)
                q2ones[b] = cols.tile([2, LQ], F16, tag="q2row", name=f"q2row_{b}")
                nc.scalar.activation(q2ones[b][:], p_q2r[:], AF.Copy)
            for b in BS:
                p_c1r = pffw.tile([2, LC], F32, tag="pf", name=f"pc1r_{b}")
                for hc in range(NHC):
                    mm(p_c1r[:], colw[:, 2 * hc:2 * hc + 2], ctxT[b][hc],
                       start=(hc == 0), stop=False)
                mm(p_c1r[:], o01, ones_row[:, 0:LC], start=False, stop=True)
                c1ones[b] = cols.tile([2, LC], F16, tag="c1row", name=f"c1row_{b}")
                nc.scalar.activation(c1ones[b][:], p_c1r[:], AF.Copy)

            ucols = {b: attn.tile([128, NIC + 2], F16, tag=f"ucols_{b}",
                                  name=f"ucols_{b}") for b in BS}
            a_n = {b: [] for b in BS}
            for ic in range(NIC):
                icsl = slice(ic * 128, (ic + 1) * 128)
                for b in BS:
                    p_sim = pmm.tile([128, LQ], F32, tag="pmm", name=f"psim_{b}{ic}")
                    for hc in range(NHC):
                        mm(p_sim[:], ctxw3T[b][hc][:, icsl], quesT[b][hc],
                           start=(hc == 0), stop=False)
                    mm(p_sim[:], c1ones[b][:, icsl], q2ones[b][:],
                       start=False, stop=True)

                    a_un = attn.tile([128, LQ], F32, tag=f"aun{ic}_{b}")
                    nc.scalar.activation(a_un[:], p_sim[:], AF.Exp)
                    ssum = cols.tile([128, 1], F32, tag="ssum", name=f"ssum_{b}{ic}")
                    nc.vector.reduce_sum(out=ssum[:], in_=a_un[:], axis=AX.X)
                    srec = cols.tile([128, 1], F32, tag="srec", name=f"srec_{b}{ic}")
                    nc.vector.reciprocal(srec[:], ssum[:])
                    nc.vector.reduce_max(out=ucols[b][:, ic:ic + 1], in_=a_un[:], axis=AX.X)
                    t_ = attn.tile([128, LQ], F16, tag=f"an{ic}_{b}")
                    nc.vector.tensor_scalar_mul(t_[:], a_un[:], srec[:])
                    a_n[b].append(t_)

            # a^T [j-part, i-free]
            aT = {b: attn.tile([LQ, LC], F16, tag=f"aT_{b}", name=f"aT_{b}")
                  for b in BS}
            for b in BS:
                for ic in range(NIC):
                    p = pffw.tile([LQ, 128], F16, tag="pf", name=f"paT_{b}{ic}")
                    nc.tensor.transpose(p[:], a_n[b][ic][:], ident)
                    nc.scalar.activation(aT[b][:, ic * 128:(ic + 1) * 128], p[:], AF.Copy)

            # softmax-over-i weights: denominator + broadcast of 1/den
            invb = {}
            for b in BS:
                ucol1 = cols.tile([128, 1], F16, tag="ucol1", name=f"ucol1_{b}")
                with nc.allow_low_precision(reason="4-term bf16 softmax-denominator partial sum"):
                    nc.vector.reduce_sum(out=ucol1[:], in_=ucols[b][:, 0:NIC], axis=AX.X)
                p_den = pat.tile([1, 2], F32, tag="pt", name=f"pden_{b}")
                mm(p_den[:], ucol1[:], ones2, start=True, stop=True)
                inv2f = cols.tile([1, 2], F32, tag="inv2f", name=f"inv2f_{b}")
                nc.vector.reciprocal(inv2f[:], p_den[:])
                inv2 = cols.tile([1, 2], F16, tag="inv2", name=f"inv2_{b}")
                nc.scalar.activation(inv2[:], inv2f[:], AF.Copy)
                p_bc = pat.tile([128, 2], F32, tag="pt", name=f"pbc_{b}")
                mm(p_bc[:], ones_row[:, 0:128], inv2[:], start=True, stop=True)
                invb[b] = cols.tile([128, 1], F32, tag="invb", name=f"invb_{b}")
                nc.scalar.activation(invb[b][:], p_bc[:, 0:1], AF.Copy)

            q2cc = {b: [] for b in BS}
            for b in BS:
                for hs in range(NHC):
                    p_q2c = pat.tile([128, 2], F32, tag="pt", name=f"pq2c_{b}{hs}")
                    for ic in range(NIC):
                        mm(p_q2c[:], ctx_sb[b][ic][:, hs * 128:(hs + 1) * 128],
                           ucols[b][:, ic:ic + 2], start=(ic == 0), stop=(ic == NIC - 1))
                    t_ = cols.tile([128, 1], F32, tag=f"q2cc{hs}", name=f"q2cc_{b}{hs}")
                    nc.vector.tensor_mul(t_[:], p_q2c[:, 0:1], invb[b][:])
                    q2cc[b].append(t_)

            # x^T chunks: 0-1 ctx^T, 2-3 c2q^T, 4-5 (ctx*c2q)^T, 6-7 (ctx*q2c)^T
            xT = {}
            for b in BS:
                xT[b] = [ctxT[b][0], ctxT[b][1]]
                for hs in range(NHC):
                    p_c2q = pffw.tile([128, LC], F32, tag="pf", name=f"pc2q_{b}{hs}")
                    mm(p_c2q[:], ques_sb[b][:, hs * 128:(hs + 1) * 128], aT[b][:],
                       start=True, stop=True)
                    t_ = acts.tile([128, LC], F16, tag=f"xT{2 + hs}_{b}")
                    nc.scalar.activation(t_[:], p_c2q[:], AF.Copy)
                    xT[b].append(t_)
                for hc in range(NHC):
                    t_ = acts.tile([128, LC], F16, tag=f"xT{4 + hc}_{b}")
                    nc.vector.tensor_mul(t_[:], ctxT[b][hc], xT[b][2 + hc][:])
                    xT[b].append(t_)
                for hc in range(NHC):
                    t_ = acts.tile([128, LC], F16, tag=f"xT{6 + hc}_{b}")
                    nc.vector.tensor_scalar_mul(t_[:], ctxT[b][hc], q2cc[b][hc][:])
                    xT[b].append(t_)
            return xT

        def ffw(b, xT):
            # Both layers' h1 group blocks run back-to-back, then both out
            # blocks: the e-layer's h1 matmuls hide the s-layer's last h1
            # evacuation latency (and vice versa for the out stages), so the
            # PE never stalls at an h1->out transition.
            h1 = {}
            dc_order = [0, 1, 2, 3, 4, 5, 6, 7]
            for lname, colb1 in (("s", COL_B1S), ("e", COL_B1E)):
                w1t = wtiles[lname][0]
                h1[lname] = []
                for fc in range(NFC):
                    p = pffw.tile([128, LC], F32, tag="pf", name=f"ph1{lname}_{b}{fc}")
                    for k, dc in enumerate(dc_order):
                        mm(p[:], w1t[:, dc, fc * 128:(fc + 1) * 128], xT[dc],
                           start=(k == 0), stop=(k == NDC - 1))
                    t_ = acts.tile([128, LC], F16, tag=f"h1{lname}{fc}",
                                   name=f"h1{lname}{fc}_{b}")
                    nc.vector.tensor_scalar(
                        out=t_[:], in0=p[:],
                        scalar1=f32p[:, colb1 + fc:colb1 + fc + 1],
                        scalar2=0.0, op0=ALU.add, op1=ALU.max)
                    h1[lname].append(t_)
            sT, eT = [], []
            for lname, colb2, dst in (("s", COL_B2S, sT), ("e", COL_B2E, eT)):
                w2t = wtiles[lname][1]
                for dc in range(NDC):
                    p = pffw.tile([128, LC], F32, tag="pf", name=f"po{lname}_{b}{dc}")
                    for fc in range(NFC):
                        mm(p[:], w2t[:, fc, dc * 128:(dc + 1) * 128], h1[lname][fc][:],
                           start=(fc == 0), stop=(fc == NFC - 1))
                    t_ = acts.tile([128, LC], F16, tag=f"{lname}T{dc}", bufs=2,
                                   name=f"{lname}T{dc}_{b}")
                    nc.scalar.activation(
                        t_[:], p[:], AF.Identity,
                        bias=f32p[:, colb2 + dc:colb2 + dc + 1],
                        scale=1.0)
                    dst.append(t_)
            return sT, eT

        def biaffine(b, sT, eT):
            # t1 rows for all three labels in one group:
            # t1[c, x] = sum_i start^T[i, x] * Wb[i, c, D]  + Wb[D, c, D]
            p_t14 = pffw.tile([4, LC], F32, tag="pf", name=f"pt14_{b}")
            for ic in range(NDC):
                mm(p_t14[:], bf16p[:, OFF_UPACK + ic * 4:OFF_UPACK + ic * 4 + 4],
                   sT[ic][:], start=(ic == 0), stop=False)
            mm(p_t14[:], wrow4, ones_row, start=False, stop=True)
            t14 = cols.tile([4, LC], F16, tag="t14", name=f"t14_{b}")
            nc.scalar.activation(t14[:], p_t14[:], AF.Copy)
            t1cols = []
            for xc in range(NIC):
                p = ptiny.tile([128, 4], F16, tag="pt", name=f"pt1c_{b}{xc}")
                nc.tensor.transpose(p[:], t14[:, xc * 128:(xc + 1) * 128],
                                    ident[0:4, 0:4])
                tsb = cols.tile([128, 4], F32, tag=f"t1c{xc}", name=f"t1c{xc}_{b}")
                nc.vector.tensor_copy(tsb[:], p[:])
                t1cols.append(tsb)

            for c in range(C):
                # t_c^T[j, x] = sum_i Wb[i,c,j] * start^T[i, x]  (+ v_c[j])
                tt = []
                for jc in range(NJC):
                    p = pmm.tile([128, LC], F32, tag="pmm", name=f"pt_{b}{c}{jc}")
                    for ic in range(NDC):
                        mm(p[:], wbt_tile[:, ic, c, jc * 128:(jc + 1) * 128], sT[ic][:],
                           start=(ic == 0), stop=(ic == NDC - 1))
                    t_ = tring.tile([128, LC], F16, tag="t", name=f"t_{b}{c}{jc}")
                    nc.vector.tensor_scalar_add(
                        t_[:], p[:],
                        f32p[:, COL_VC + c * NJC + jc:COL_VC + c * NJC + jc + 1])
                    tt.append(t_)

                # score_c[x, y] = sum_j t_c^T[j, x] * end^T[j, y] + t1_c[x],
                # accumulated into a [128, 4, LC] bf16 plane group, DMA'd out
                # as one [LC, LC] label plane on the scalar hardware queue.
                planes = oplane.tile([128, NIC, LC], F16, tag="opl",
                                     name=f"opl_{b}{c}")
                last = (b == NB - 1 and c == C - 1)
                for xc in range(NIC):
                    if last and xc == NIC - 1:
                        # final plane chunk in two halves: half A's evac+DMA
                        # overlap half B's matmuls, halving the tail chain.
                        for h in range(2):
                            ysl = slice(h * 256, (h + 1) * 256)
                            ph = pmm.tile([128, 256], F32, tag="pmm",
                                          name=f"ps_{b}{c}{xc}{h}")
                            for jc in range(NJC):
                                mm(ph[:], tt[jc][:, xc * 128:(xc + 1) * 128],
                                   eT[jc][:, ysl], start=(jc == 0),
                                   stop=(jc == NJC - 1))
                            if h == 0:
                                nc.scalar.activation(
                                    planes[:, xc, 0:256], ph[:], AF.Identity,
                                    bias=t1cols[xc][:, c:c + 1], scale=1.0)
                            else:
                                nc.vector.tensor_scalar_add(
                                    planes[:, xc, 256:512], ph[:],
                                    t1cols[xc][:, c:c + 1])
                            nc.sync.dma_start(
                                out=out_d[b, c, :, xc, ysl],
                                in_=planes[:, xc, ysl])
                        continue
                    p = pmm.tile([128, LC], F32, tag="pmm", name=f"ps_{b}{c}{xc}")
                    for jc in range(NJC):
                        mm(p[:], tt[jc][:, xc * 128:(xc + 1) * 128], eT[jc][:],
                           start=(jc == 0), stop=(jc == NJC - 1))
                    if last and xc % 2 == 1:
                        nc.vector.tensor_scalar_add(planes[:, xc, :], p[:],
                                                    t1cols[xc][:, c:c + 1])
                    else:
                        nc.scalar.activation(planes[:, xc, :], p[:], AF.Identity,
                                             bias=t1cols[xc][:, c:c + 1], scale=1.0)
                    if last:
                        nc.sync.dma_start(out=out_d[b, c, :, xc, :],
                                          in_=planes[:, xc, :])
                if not last:
                    nc.sync.dma_start(out=out_d[b, c], in_=planes[:])

        # ---- phase-interleaved schedule ----
        # A0+A1 interleaved, then both FFWs, then both biaffines (sT/eT are
        # double-buffered), so the PE stream never stalls on front-end work
        # mid-kernel.
        xT = attention_pair()
        se0 = ffw(0, xT[0])
        se1 = ffw(1, xT[1])
        biaffine(0, *se0)
        biaffine(1, *se1)


_PROGRAM_CACHE = {}


def _get_program():
    if "nc" not in _PROGRAM_CACHE:
        _PROGRAM_CACHE["nc"] = _build_program()
    return _PROGRAM_CACHE["nc"]


def _pack_host_inputs(w_sim, W1s, b1s, W2s, b2s, W1e, b1e, W2e, b2e, Wb):
    """Build the shared (replicated) input arrays from the raw weights."""
    import ml_dtypes
    f32, f16 = np.float32, ml_dtypes.bfloat16
    w1, w2, w3 = [np.asarray(w_sim[k * H:(k + 1) * H], f32) for k in range(3)]

    f32p = np.zeros((128, NF32), f32)
    for hc in range(NHC):
        f32p[:, COL_W3 + hc] = w3[hc * 128:(hc + 1) * 128]
    for fc in range(NFC):
        f32p[:, COL_B1S + fc] = b1s[fc * 128:(fc + 1) * 128]
        f32p[:, COL_B1E + fc] = b1e[fc * 128:(fc + 1) * 128]
    for dc in range(NDC):
        f32p[:, COL_B2S + dc] = b2s[dc * 128:(dc + 1) * 128]
        f32p[:, COL_B2E + dc] = b2e[dc * 128:(dc + 1) * 128]
    for c in range(C):
        for jc in range(NJC):
            f32p[:, COL_VC + c * NJC + jc] = Wb[D, c, jc * 128:(jc + 1) * 128]

    bf16p = np.zeros((128, NBF16), f32)
    bf16p[:, OFF_ONES:OFF_ONES + 512] = 1.0
    for hc in range(NHC):
        bf16p[:, OFF_COLW + 2 * hc] = w1[hc * 128:(hc + 1) * 128]
        bf16p[:, OFF_COLW + 5 + 2 * hc] = w2[hc * 128:(hc + 1) * 128]
    bf16p[:, OFF_Z] = 1.0
    bf16p[:, OFF_Z + 2] = 1.0
    bf16p[:, OFF_IDENT:OFF_IDENT + 128] = np.eye(128, dtype=f32)
    for dc in range(NDC):
        for c in range(C):
            bf16p[:, OFF_UPACK + dc * 4 + c] = Wb[dc * 128:(dc + 1) * 128, c, D]
    bf16p[:, OFF_WROW4:OFF_WROW4 + C] = Wb[D, :, D][None, :]

    def pmaj(a, nchunk):
        # [nchunk*128, F...] -> [128, nchunk, F...] (SBUF-tile layout)
        return np.ascontiguousarray(
            a.reshape((nchunk, 128) + a.shape[1:]).swapaxes(0, 1))

    return {
        "f32pack": f32p,
        "bf16pack": bf16p.astype(f16),
        "wb": pmaj(Wb[:D].astype(f16), NDC),
        "W1s": pmaj(W1s.astype(f16), NDC),
        "W2s": pmaj(W2s.astype(f16), NFC),
        "W1e": pmaj(W1e.astype(f16), NDC),
        "W2e": pmaj(W2e.astype(f16), NFC),
    }


def kernel(ctx_emb, ques_emb, w_sim, W1s, b1s, W2s, b2s, W1e, b1e, W2e, b2e, Wb,
           _trace=False, _tmpdir=None):
    from concourse.bass_utils import run_bass_kernel_spmd
    import ml_dtypes

    # accept jax/np arrays of any layout
    (ctx_emb, ques_emb, w_sim, W1s, b1s, W2s, b2s, W1e, b1e, W2e, b2e, Wb) = (
        np.asarray(a, dtype=np.float32)
        for a in (ctx_emb, ques_emb, w_sim, W1s, b1s, W2s, b2s, W1e, b1e, W2e,
                  b2e, Wb))

    nc = _get_program()
    shared = _pack_host_inputs(w_sim, W1s, b1s, W2s, b2s, W1e, b1e, W2e, b2e, Wb)
    ctx16 = ctx_emb.astype(ml_dtypes.bfloat16)
    ques16 = np.ascontiguousarray(ques_emb.astype(ml_dtypes.bfloat16))
    # p-major repacks matching the SBUF tile layouts ([.., 128, chunk, free])
    ctxp = np.ascontiguousarray(
        ctx16.reshape(B, NIC, 128, H).swapaxes(1, 2))            # [B,128,4,H]
    ctxTp = np.ascontiguousarray(
        ctx16.transpose(0, 2, 1).reshape(B, NHC, 128, LC).swapaxes(1, 2))
    quesTp = np.ascontiguousarray(
        ques16.transpose(0, 2, 1).reshape(B, NHC, 128, LQ).swapaxes(1, 2))
    in_maps = []
    for core in range(N_CORES):
        sl = slice(core * NB, (core + 1) * NB)
        in_maps.append({"ctx": ctxp[sl], "ques": ques16[sl],
                        "ctxT": ctxTp[sl], "quesT": quesTp[sl], **shared})

    kw = {}
    if _trace:
        kw = {"trace": True, "tmpdir": _tmpdir}
    res = run_bass_kernel_spmd(nc, in_maps, list(range(N_CORES)), **kw)
    # device layout is [NB, C, 128, NIC, LC] bf16 (p-major planes);
    # upcast + unpermute to [B, LC, LC, C] on the host.
    outs = []
    for i in range(N_CORES):
        o = np.asarray(res.results[i]["out"])  # [NB, C, 128, NIC, LC]
        o = o.astype(np.float32).transpose(0, 3, 2, 4, 1)  # [NB, NIC, 128, LC, C]
        outs.append(o.reshape(NB, LC, LC, C))
    out = np.ascontiguousarray(np.concatenate(outs, axis=0))
    if _trace:
        return out, res
    return out

